# revision 27
# baseline (speedup 1.0000x reference)
"""Trainium2 Bass kernel for nn_NodeModel (GNN message passing).

  out = relu(concat([x, scatter_mean(edge_attr, col), u[batch]]) @ W1 + b1) @ W2 + b2

Strategy (8 NeuronCores, data-parallel over destination nodes):
  * Nodes are partitioned contiguously across the 8 cores (12500/core);
    edges live with their destination node, so scatter_mean is a purely
    local segment reduction (no cross-core traffic).
  * Within a core, nodes are permuted in degree-descending order and
    grouped into 100 windows of 128 node slots. Each window w is padded
    to cap[w] = max degree in that window (rounded up to a multiple of
    2, shared across cores) -- ~3% padding instead of the 2x a global
    max-degree pad costs. Edge values and x ship as fp8 (e3m4); the
    1/count scaling of scatter_mean is applied on device as a
    per-partition activation scale, so quantization happens at the
    natural ~N(0,1) scale of edge_attr.
  * u[batch] is never materialized: host precomputes W1u_eff =
    u @ W1[80:144] (exact, f32) and ships a 0/1 one-hot graph-membership
    matrix in fp8 (exact). Its contribution enters the hidden-layer
    PSUM as one extra matmul W1u_eff.T @ onehot.
  * Device, per core and per window: DMA the fp8 edge block
    [128, 16*cap], DVE-reduce over the cap axis, scale by 1/count,
    PE-transpose to [16, 128]. Per group of 4 windows: psum
    [128H, 512] = W1e.T@eT + W1x.T@xT + W1u_eff.T@onehot, ReLU+bias,
    [64, 512] = W2.T@hid, +bias, DMA out in f16.
  * The work is cut into 5 pipeline stages of 20 windows each; every
    stage's inputs ship as one contiguous uint8 blob per core (the
    program bitcasts slices of it). Stage puts are queued
    asynchronously and stage outputs are fetched on worker threads, so
    output D2H overlaps later input H2D (the link is full duplex).
"""

import numpy as np
import ml_dtypes
from concurrent.futures import ThreadPoolExecutor

_BF16 = np.dtype(ml_dtypes.bfloat16)
_FP8E3 = np.dtype(ml_dtypes.float8_e3m4)

F_E, F_X, F_U, H, F_OUT = 16, 64, 64, 128, 64
N_NODES, N_GRAPHS = 100000, 64
NC, NPC, WPC, B = 8, 12500, 100, 4
SLOTS = WPC * 128          # 12800 node slots per core
NB = WPC // B              # 25 MLP groups per core
STAGES = 5
WPS = WPC // STAGES        # 20 windows per stage
GPS = NB // STAGES         # 5 MLP groups per stage
SPS = WPS * 128            # 2560 slots per stage
XT_FP8 = True              # ship x in fp8e3 instead of bf16

_PROGRAM_CACHE = {}
_RUNNER_CACHE = {}


def _align(n, a=64):
    return (n + a - 1) // a * a


def _blob_layout(caps_k):
    """Byte offsets of each tensor inside a stage blob."""
    xsz = 1 if XT_FP8 else 2
    sizes = [
        ("edges", int(sum(caps_k)) * 128 * F_E),
        ("xt", F_X * SPS * xsz),
        ("gid", SPS * 2),
        ("invc", 128 * WPS * 4),
        ("w1x", F_X * H * 2),
        ("w1e", F_E * H * 2),
        ("w1u", N_GRAPHS * H * 2),
        ("w2", H * F_OUT * 2),
        ("b1", H * 4),
        ("b2", F_OUT * 4),
        ("ident", 128 * 128 * 4),
        ("iota64", N_GRAPHS * 4),
        ("ones", N_GRAPHS * 2),
    ]
    offs, cur = {}, 0
    for name, sz in sizes:
        offs[name] = cur
        cur = _align(cur + sz)
    return offs, cur


# ---------------------------------------------------------------- host side
def _plan_and_preprocess(inputs):
    x = np.asarray(inputs["x"], np.float32)
    ea = np.asarray(inputs["edge_attr"], np.float32)
    u = np.asarray(inputs["u"], np.float32)
    W1 = np.asarray(inputs["W1"], np.float32)
    b1 = np.asarray(inputs["b1"], np.float32)
    W2 = np.asarray(inputs["W2"], np.float32)
    b2 = np.asarray(inputs["b2"], np.float32)
    col = np.asarray(np.asarray(inputs["edge_index"])[1], np.int64)
    batch = np.asarray(inputs["batch"], np.int64)

    N, E = x.shape[0], col.shape[0]
    assert N == NC * NPC, (N, NC, NPC)

    cnt = np.bincount(col, minlength=N)
    invc = (1.0 / np.maximum(cnt, 1)).astype(np.float32)

    # per-core degree-descending node permutation; shared window caps
    cnt2 = cnt.reshape(NC, NPC)
    order = np.argsort(-cnt2, axis=1, kind="stable")          # [NC, NPC]
    slot_of_local = np.empty((NC, NPC), np.int64)
    np.put_along_axis(slot_of_local, order,
                      np.broadcast_to(np.arange(NPC), (NC, NPC)), axis=1)
    deg_sorted = np.take_along_axis(cnt2, order, axis=1)
    padded = np.zeros((NC, SLOTS), np.int64)
    padded[:, :NPC] = deg_sorted
    caps = padded.reshape(NC, WPC, 128).max(axis=2).max(axis=0)
    caps = np.maximum(caps, 2)
    caps = ((caps + 1) // 2 * 2).astype(np.int64)             # [WPC]

    offs = np.zeros(WPC, np.int64)
    offs[1:] = np.cumsum(caps[:-1]) * (128 * F_E)
    total = int(caps.sum()) * 128 * F_E                        # bytes per core

    # edge scatter into per-core flat fp8 arrays (window blocks [128, 16, cap])
    order_e = np.argsort(col, kind="stable")
    cols = col[order_e]
    eas8 = ea[order_e].astype(_FP8E3)
    starts = np.concatenate([[0], np.cumsum(cnt)[:-1]])
    rank = np.arange(E, dtype=np.int64) - starts[cols]
    c_of = cols // NPC
    s_of = slot_of_local[c_of, cols - c_of * NPC]
    w_of = s_of >> 7
    p_of = s_of & 127
    capw = caps[w_of]
    base = (c_of * total + offs[w_of] + p_of * (F_E * capw) + rank).astype(np.int32)
    cap32 = capw.astype(np.int32)
    A = np.zeros(NC * total, _FP8E3)
    for f in range(F_E):
        A[base + np.int32(f) * cap32] = eas8[:, f]
    A = A.reshape(NC, total)

    # node features transposed into slot order
    rows = np.arange(NC)[:, None]
    xdt = _FP8E3 if XT_FP8 else _BF16
    xp = np.zeros((NC, SLOTS, F_X), xdt)
    xp[rows, slot_of_local] = x.reshape(NC, NPC, F_X).astype(xdt)
    xt = np.ascontiguousarray(xp.transpose(0, 2, 1))           # [NC, 64, SLOTS]

    # graph id per slot (pad slots get -1 -> no one-hot match -> zero u-term)
    gid = np.full((NC, SLOTS), -1.0, _BF16)
    gid[rows, slot_of_local] = batch.reshape(NC, NPC).astype(_BF16)

    iv = np.ones((NC, SLOTS), np.float32)
    iv[rows, slot_of_local] = invc.reshape(NC, NPC)
    ivt = np.ascontiguousarray(
        iv.reshape(NC, WPC, 128).transpose(0, 2, 1))           # [NC, 128, WPC]

    w1x = np.ascontiguousarray(W1[0:F_X], dtype=_BF16)                 # [64,128]
    w1e = np.ascontiguousarray(W1[F_X:F_X + F_E], dtype=_BF16)         # [16,128]
    w1u = np.ascontiguousarray(u @ W1[F_X + F_E:], dtype=_BF16)        # [64,128]
    w2 = np.ascontiguousarray(W2, dtype=_BF16)                         # [128,64]
    b1c = np.ascontiguousarray(b1, np.float32)
    b2c = np.ascontiguousarray(b2, np.float32)
    identc = np.eye(128, dtype=np.float32)
    iotac = np.arange(N_GRAPHS, dtype=np.float32)
    onesc = np.ones(N_GRAPHS, _BF16)
    wbytes = [w1x, w1e, w1u, w2, b1c, b2c, identc, iotac, onesc]

    stage_caps = [tuple(int(c) for c in caps[k * WPS:(k + 1) * WPS])
                  for k in range(STAGES)]
    in_maps = [{} for _ in range(NC)]
    for k in range(STAGES):
        layout, nbytes = _blob_layout(stage_caps[k])
        e0 = int(offs[k * WPS])
        e1 = e0 + int(sum(stage_caps[k])) * 128 * F_E
        s0, s1 = k * SPS, (k + 1) * SPS
        for ci in range(NC):
            blob = np.zeros(nbytes, np.uint8)

            def put(name, arr):
                bts = np.ascontiguousarray(arr).view(np.uint8).ravel()
                blob[layout[name]:layout[name] + bts.size] = bts

            put("edges", A[ci, e0:e1])
            put("xt", xt[ci][:, s0:s1])
            put("gid", gid[ci][s0:s1])
            put("invc", ivt[ci][:, k * WPS:(k + 1) * WPS])
            for nm, arr in zip(("w1x", "w1e", "w1u", "w2", "b1", "b2", "ident",
                                "iota64", "ones"),
                               wbytes):
                put(nm, arr)
            in_maps[ci][f"blob{k}"] = blob

    plan = dict(stage_caps=tuple(stage_caps), slot_of_local=slot_of_local)
    return plan, in_maps


def _postprocess(stage_outs, plan):
    """stage_outs: list of STAGES arrays [NC*GPS, F_OUT, 512] f16."""
    slot_of_local = plan["slot_of_local"]
    out = np.empty((NC * NPC, F_OUT), np.float32)
    o = np.concatenate(
        [so.reshape(NC, GPS, F_OUT, B * 128) for so in stage_outs], axis=1)
    for ci in range(NC):
        o2 = o[ci].transpose(1, 0, 2).reshape(F_OUT, SLOTS)
        out[ci * NPC:(ci + 1) * NPC] = o2[:, slot_of_local[ci]].T
    return out


# ------------------------------------------------------------- device side
def _build_stage(caps_k):
    import concourse.bacc as bacc
    import concourse.mybir as mybir
    import concourse.tile as tile
    from contextlib import ExitStack

    f32 = mybir.dt.float32
    bf16 = mybir.dt.bfloat16
    f16 = mybir.dt.float16
    fp8 = mybir.dt.float8e3
    u8 = mybir.dt.uint8
    AF = mybir.ActivationFunctionType

    caps_k = list(caps_k)
    layout, nbytes = _blob_layout(caps_k)
    eoffs = [0] * WPS
    for w in range(1, WPS):
        eoffs[w] = eoffs[w - 1] + caps_k[w - 1] * 128 * F_E

    nc = bacc.Bacc("TRN2", target_bir_lowering=False)
    blob_d = nc.dram_tensor("blob", [nbytes], u8, kind="ExternalInput")
    out_d = nc.dram_tensor("outT", [GPS, F_OUT, B * 128], f16,
                           kind="ExternalOutput")

    def view(name, dt, p, q):
        o = layout[name]
        sz = p * q * np.dtype(mybir.dt.np(dt)).itemsize
        return blob_d[o:o + sz].bitcast(dt).rearrange("(p q) -> p q", p=p)

    with tile.TileContext(nc) as tc, ExitStack() as ctx:
        consts = ctx.enter_context(tc.tile_pool(name="consts", bufs=1))
        edge_pool = ctx.enter_context(tc.tile_pool(name="edges", bufs=4))
        gsn_pool = ctx.enter_context(tc.tile_pool(name="gsn", bufs=4))
        gsc_pool = ctx.enter_context(tc.tile_pool(name="gsc", bufs=2))
        ea_pool = ctx.enter_context(tc.tile_pool(name="ea", bufs=2))
        hid_pool = ctx.enter_context(tc.tile_pool(name="hid", bufs=2))
        out_pool = ctx.enter_context(tc.tile_pool(name="outs", bufs=3))
        pse_pool = ctx.enter_context(
            tc.tile_pool(name="pse", bufs=2, space="PSUM"))
        psh_pool = ctx.enter_context(
            tc.tile_pool(name="psh", bufs=2, space="PSUM"))
        pso_pool = ctx.enter_context(
            tc.tile_pool(name="pso", bufs=2, space="PSUM"))
        psb_pool = ctx.enter_context(
            tc.tile_pool(name="psb", bufs=1, space="PSUM"))

        w1x_t = consts.tile([F_X, H], bf16)
        nc.sync.dma_start(w1x_t[:], view("w1x", bf16, F_X, H))
        w1e_t = consts.tile([F_E, H], bf16)
        nc.sync.dma_start(w1e_t[:], view("w1e", bf16, F_E, H))
        w1u_t = consts.tile([N_GRAPHS, H], bf16)
        nc.sync.dma_start(w1u_t[:], view("w1u", bf16, N_GRAPHS, H))
        w2_t = consts.tile([H, F_OUT], bf16)
        nc.sync.dma_start(w2_t[:], view("w2", bf16, H, F_OUT))
        b1_t = consts.tile([H, 1], f32)
        nc.sync.dma_start(b1_t[:], view("b1", f32, H, 1))
        b2_t = consts.tile([F_OUT, 1], f32)
        nc.sync.dma_start(b2_t[:], view("b2", f32, F_OUT, 1))
        invc_t = consts.tile([128, WPS], f32)
        nc.sync.dma_start(invc_t[:], view("invc", f32, 128, WPS))

        if XT_FP8:
            xt8_t = consts.tile([F_X, SPS], fp8)
            nc.sync.dma_start(xt8_t[:], view("xt", fp8, F_X, SPS))
            xt_t = consts.tile([F_X, SPS], bf16)
            nc.vector.tensor_copy(xt_t[:], xt8_t[:])
        else:
            xt_t = consts.tile([F_X, SPS], bf16)
            nc.sync.dma_start(xt_t[:], view("xt", bf16, F_X, SPS))
        ident_t = consts.tile([128, 128], f32)
        nc.sync.dma_start(ident_t[:], view("ident", f32, 128, 128))

        # one-hot graph membership built on device: broadcast gid across 64
        # partitions with a K=1 matmul, then compare against iota(64)
        gid_t = consts.tile([1, SPS], bf16)
        nc.sync.dma_start(gid_t[:], view("gid", bf16, 1, SPS))
        iota_t = consts.tile([N_GRAPHS, 1], f32)
        nc.sync.dma_start(iota_t[:], view("iota64", f32, N_GRAPHS, 1))
        ones_t = consts.tile([1, N_GRAPHS], bf16)
        nc.sync.dma_start(ones_t[:], view("ones", bf16, 1, N_GRAPHS))
        gidb_t = consts.tile([N_GRAPHS, SPS], f32)
        ohb_t = consts.tile([N_GRAPHS, SPS], bf16)
        for c in range(SPS // 512):
            psb = psb_pool.tile([N_GRAPHS, 512], f32)
            nc.tensor.matmul(psb[:], ones_t[:], gid_t[:, c * 512:(c + 1) * 512],
                             start=True, stop=True)
            nc.vector.tensor_copy(gidb_t[:, c * 512:(c + 1) * 512], psb[:])
        nc.vector.tensor_scalar(
            out=ohb_t[:], in0=gidb_t[:], scalar1=iota_t[:], scalar2=None,
            op0=mybir.AluOpType.is_equal,
        )

        eb = layout["edges"]
        for g in range(GPS):
            gsc = gsc_pool.tile([128, B * F_E], f32)
            for j in range(B):
                w = g * B + j
                cw = caps_k[w]
                et = edge_pool.tile([128, F_E * cw], fp8)
                src = blob_d[eb + eoffs[w]:eb + eoffs[w] + 128 * F_E * cw]
                nc.sync.dma_start(
                    et[:], src.bitcast(fp8).rearrange("(p q) -> p q", p=128))
                gsn = gsn_pool.tile([128, F_E], f32)
                nc.vector.tensor_reduce(
                    out=gsn[:],
                    in_=et[:].rearrange("p (f e) -> p f e", e=cw),
                    axis=mybir.AxisListType.X,
                    op=mybir.AluOpType.add,
                )
                nc.scalar.activation(
                    gsc[:, j * F_E:(j + 1) * F_E], gsn[:], AF.Identity,
                    scale=invc_t[:, w:w + 1],
                )

            pse = pse_pool.tile([F_E, B * 128], f32)
            for j in range(B):
                nc.tensor.transpose(
                    pse[:, j * 128:(j + 1) * 128],
                    gsc[:, j * F_E:(j + 1) * F_E],
                    ident_t[:],
                )
            ea = ea_pool.tile([F_E, B * 128], bf16)
            nc.vector.tensor_copy(ea[:], pse[:])

            psh = psh_pool.tile([H, B * 128], f32)
            nc.tensor.matmul(psh[:], w1e_t[:], ea[:], start=True, stop=False)
            nc.tensor.matmul(psh[:], w1x_t[:],
                             xt_t[:, g * 512:(g + 1) * 512],
                             start=False, stop=False)
            nc.tensor.matmul(psh[:], w1u_t[:],
                             ohb_t[:, g * 512:(g + 1) * 512],
                             start=False, stop=True)
            hid = hid_pool.tile([H, B * 128], bf16)
            nc.scalar.activation(hid[:], psh[:], AF.Relu, bias=b1_t[:])

            pso = pso_pool.tile([F_OUT, B * 128], f32)
            nc.tensor.matmul(pso[:], w2_t[:], hid[:], start=True, stop=True)
            outs = out_pool.tile([F_OUT, B * 128], f16)
            nc.scalar.activation(outs[:], pso[:], AF.Identity, bias=b2_t[:])
            nc.sync.dma_start(out_d[g], outs[:])

    nc.finalize()
    return nc


def _get_program(caps_k):
    key = (caps_k, XT_FP8)
    if key not in _PROGRAM_CACHE:
        _PROGRAM_CACHE[key] = _build_stage(caps_k)
    return _PROGRAM_CACHE[key]


# ------------------------------------------------------------- runner
class _Stage:
    def __init__(self, nc, sharding, mesh, jax, bass2jax, mybir):
        partition_name = (nc.partition_id_tensor.name
                          if nc.partition_id_tensor else None)
        in_names, out_names, out_avals = [], [], []
        for alloc in nc.m.functions[0].allocations:
            if not isinstance(alloc, mybir.MemoryLocationSet):
                continue
            name = alloc.memorylocations[0].name
            if alloc.kind == "ExternalInput":
                if name != partition_name:
                    in_names.append(name)
            elif alloc.kind == "ExternalOutput":
                out_names.append(name)
                out_avals.append(jax.core.ShapedArray(
                    tuple(alloc.tensor_shape), mybir.dt.np(alloc.dtype)))
        in_names_all = list(in_names) + out_names
        if partition_name is not None:
            in_names_all.append(partition_name)

        def _body(*args):
            operands = list(args)
            if partition_name is not None:
                operands.append(bass2jax.partition_id_tensor())
            return tuple(bass2jax._bass_exec_p.bind(
                *operands,
                out_avals=tuple(out_avals),
                in_names=tuple(in_names_all),
                out_names=tuple(out_names),
                lowering_input_output_aliases=(),
                sim_require_finite=True,
                sim_require_nnan=True,
                nc=nc,
            ))

        from jax.sharding import PartitionSpec
        from jax.experimental.shard_map import shard_map
        n_ops = len(in_names) + len(out_names)
        self.sharded = jax.jit(
            shard_map(_body, mesh=mesh,
                      in_specs=(PartitionSpec("core"),) * n_ops,
                      out_specs=(PartitionSpec("core"),) * len(out_names),
                      check_rep=False),
            keep_unused=True,
        )
        # output-named dummy operands: never read by the NEFF (the kernel
        # writes every output element), device-resident, reused every call
        self.dummy_outs = [
            jax.device_put(np.zeros((NC * a.shape[0], *a.shape[1:]), a.dtype),
                           sharding)
            for a in out_avals
        ]


class _Runner:
    def __init__(self, plan):
        import jax
        from jax.sharding import Mesh, PartitionSpec, NamedSharding
        import concourse.bass2jax as bass2jax
        import concourse.mybir as mybir

        self.jax = jax
        bass2jax.install_neuronx_cc_hook()
        devices = jax.devices()[:NC]
        mesh = Mesh(np.asarray(devices), ("core",))
        self.sharding = NamedSharding(mesh, PartitionSpec("core"))
        self.stages = [
            _Stage(_get_program(plan["stage_caps"][k]), self.sharding, mesh,
                   jax, bass2jax, mybir)
            for k in range(STAGES)
        ]
        self.pool = ThreadPoolExecutor(2)
        self.jax.block_until_ready(
            [d for s in self.stages for d in s.dummy_outs])

    def call(self, in_maps):
        """Preprocessed per-core blobs -> per-stage host output arrays."""
        futs = []
        for k in range(STAGES):
            g = np.concatenate([in_maps[ci][f"blob{k}"] for ci in range(NC)])
            d = self.jax.device_put(g, self.sharding)           # async H2D
            o = self.stages[k].sharded(d, *self.stages[k].dummy_outs)
            futs.append(self.pool.submit(np.asarray, o[0]))     # D2H thread
        return [f.result() for f in futs]


def _get_runner(plan):
    key = (plan["stage_caps"], XT_FP8)
    if key not in _RUNNER_CACHE:
        _RUNNER_CACHE[key] = _Runner(plan)
    return _RUNNER_CACHE[key]


def run(inputs):
    plan, in_maps = _plan_and_preprocess(inputs)
    runner = _get_runner(plan)
    stage_outs = runner.call(in_maps)
    return _postprocess(stage_outs, plan), plan, in_maps, runner


def kernel(**inputs):
    return run(inputs)[0]


# revision 33
# speedup vs baseline: 1.2515x; 1.2515x over previous
"""Trainium2 Bass kernel for nn_NodeModel (GNN message passing).

  out = relu(concat([x, scatter_mean(edge_attr, col), u[batch]]) @ W1 + b1) @ W2 + b2

Strategy (8 NeuronCores, data-parallel over destination nodes):
  * Nodes are partitioned contiguously across the 8 cores (12500/core);
    edges live with their destination node, so scatter_mean is a purely
    local segment reduction (no cross-core traffic).
  * Within a core, nodes are permuted in degree-descending order and
    grouped into 100 windows of 128 node slots. Each window w is padded
    to cap[w] = max degree in that window (rounded up to a multiple of
    2, shared across cores) -- ~3% padding instead of the 2x a global
    max-degree pad costs. Edge values and x ship as fp8 (e3m4); the
    1/count scaling of scatter_mean is applied on device as a
    per-partition activation scale, so quantization happens at the
    natural ~N(0,1) scale of edge_attr.
  * u[batch] is never materialized: host precomputes W1u_eff =
    u @ W1[80:144] (exact, f32) and ships a 0/1 one-hot graph-membership
    matrix in fp8 (exact). Its contribution enters the hidden-layer
    PSUM as one extra matmul W1u_eff.T @ onehot.
  * Device, per core and per window: DMA the fp8 edge block
    [128, 16*cap], DVE-reduce over the cap axis, scale by 1/count,
    PE-transpose to [16, 128]. Per group of 4 windows: psum
    [128H, 512] = W1e.T@eT + W1x.T@xT + W1u_eff.T@onehot, ReLU+bias,
    [64, 512] = W2.T@hid, +bias, DMA out in f16.
  * The work is cut into 5 pipeline stages of 20 windows each; every
    stage's inputs ship as one contiguous uint8 blob per core (the
    program bitcasts slices of it). Stage puts are queued
    asynchronously and stage outputs are fetched on worker threads, so
    output D2H overlaps later input H2D (the link is full duplex).
"""

import numpy as np
import ml_dtypes
from concurrent.futures import ThreadPoolExecutor

_BF16 = np.dtype(ml_dtypes.bfloat16)
_FP8E3 = np.dtype(ml_dtypes.float8_e3m4)

F_E, F_X, F_U, H, F_OUT = 16, 64, 64, 128, 64
N_NODES, N_GRAPHS = 100000, 64
NC, NPC, WPC, B = 8, 12500, 100, 4
SLOTS = WPC * 128          # 12800 node slots per core
NB = WPC // B              # 25 MLP groups per core
STAGES = 5
WPS = WPC // STAGES        # 20 windows per stage
GPS = NB // STAGES         # 5 MLP groups per stage
SPS = WPS * 128            # 2560 slots per stage
XT_FP8 = True              # ship x in fp8e3 instead of bf16
EDGE_MASK_BITS = 3         # round edge fp8 mantissa to (4 - n) bits; the
                           # zeroed low bits make the wire's zstd ~1.3x denser

_PROGRAM_CACHE = {}
_RUNNER_CACHE = {}


def _roundmask_fp8(vals_f32, nbits):
    """fp8e3 encode with round-to-nearest onto the 2^nbits-coarser lattice.
    Bit-increment carries propagate correctly through IEEE-style encodings;
    |v| stays far below e3m4 max normal, so the carry never reaches sign."""
    b = vals_f32.astype(_FP8E3).view(np.uint8)
    if nbits == 0:
        return b.view(_FP8E3)
    unit = np.uint8(1 << nbits)
    half = np.uint8(1 << (nbits - 1))
    keep = np.uint8(0xFF ^ (unit - 1))
    frac = b & np.uint8(unit - 1)
    out = (b & keep) + np.where(frac >= half, unit, np.uint8(0)).astype(np.uint8)
    return out.view(_FP8E3)


def _align(n, a=64):
    return (n + a - 1) // a * a


def _blob_layout(caps_k):
    """Byte offsets of each tensor inside a stage blob."""
    xsz = 1 if XT_FP8 else 2
    sizes = [
        ("edges", int(sum(caps_k)) * 128 * F_E),
        ("xt", F_X * SPS * xsz),
        ("gid", SPS * 2),
        ("invc", 128 * WPS * 4),
        ("w1x", F_X * H * 2),
        ("w1e", F_E * H * 2),
        ("w1u", N_GRAPHS * H * 2),
        ("w2", H * F_OUT * 2),
        ("b1", H * 4),
        ("b2", F_OUT * 4),
        ("iota64", N_GRAPHS * 4),
        ("ones", N_GRAPHS * 2),
        ("iotar", 128 * 2),
        ("ones128", 128 * 2),
        ("iotac", 128 * 4),
    ]
    offs, cur = {}, 0
    for name, sz in sizes:
        offs[name] = cur
        cur = _align(cur + sz)
    return offs, cur


# ---------------------------------------------------------------- host side
def _plan_and_preprocess(inputs):
    x = np.asarray(inputs["x"], np.float32)
    ea = np.asarray(inputs["edge_attr"], np.float32)
    u = np.asarray(inputs["u"], np.float32)
    W1 = np.asarray(inputs["W1"], np.float32)
    b1 = np.asarray(inputs["b1"], np.float32)
    W2 = np.asarray(inputs["W2"], np.float32)
    b2 = np.asarray(inputs["b2"], np.float32)
    col = np.asarray(np.asarray(inputs["edge_index"])[1], np.int64)
    batch = np.asarray(inputs["batch"], np.int64)

    N, E = x.shape[0], col.shape[0]
    assert N == NC * NPC, (N, NC, NPC)

    cnt = np.bincount(col, minlength=N)
    invc = (1.0 / np.maximum(cnt, 1)).astype(np.float32)

    # per-core degree-descending node permutation; shared window caps
    cnt2 = cnt.reshape(NC, NPC)
    order = np.argsort(-cnt2, axis=1, kind="stable")          # [NC, NPC]
    slot_of_local = np.empty((NC, NPC), np.int64)
    np.put_along_axis(slot_of_local, order,
                      np.broadcast_to(np.arange(NPC), (NC, NPC)), axis=1)
    deg_sorted = np.take_along_axis(cnt2, order, axis=1)
    padded = np.zeros((NC, SLOTS), np.int64)
    padded[:, :NPC] = deg_sorted
    caps = padded.reshape(NC, WPC, 128).max(axis=2).max(axis=0)
    caps = np.maximum(caps, 2)
    caps = ((caps + 1) // 2 * 2).astype(np.int64)             # [WPC]

    offs = np.zeros(WPC, np.int64)
    offs[1:] = np.cumsum(caps[:-1]) * (128 * F_E)
    total = int(caps.sum()) * 128 * F_E                        # bytes per core

    # edge scatter into per-core flat fp8 arrays (window blocks [128, 16, cap])
    order_e = np.argsort(col, kind="stable")
    cols = col[order_e]
    eas8 = _roundmask_fp8(ea[order_e], EDGE_MASK_BITS)
    starts = np.concatenate([[0], np.cumsum(cnt)[:-1]])
    rank = np.arange(E, dtype=np.int64) - starts[cols]
    c_of = cols // NPC
    s_of = slot_of_local[c_of, cols - c_of * NPC]
    w_of = s_of >> 7
    p_of = s_of & 127
    capw = caps[w_of]
    base = (c_of * total + offs[w_of] + p_of * (F_E * capw) + rank).astype(np.int32)
    cap32 = capw.astype(np.int32)
    A = np.zeros(NC * total, _FP8E3)
    for f in range(F_E):
        A[base + np.int32(f) * cap32] = eas8[:, f]
    A = A.reshape(NC, total)

    # node features transposed into slot order
    rows = np.arange(NC)[:, None]
    xdt = _FP8E3 if XT_FP8 else _BF16
    xp = np.zeros((NC, SLOTS, F_X), xdt)
    xp[rows, slot_of_local] = x.reshape(NC, NPC, F_X).astype(xdt)
    xt = np.ascontiguousarray(xp.transpose(0, 2, 1))           # [NC, 64, SLOTS]

    # graph id per slot (pad slots get -1 -> no one-hot match -> zero u-term)
    gid = np.full((NC, SLOTS), -1.0, _BF16)
    gid[rows, slot_of_local] = batch.reshape(NC, NPC).astype(_BF16)

    iv = np.ones((NC, SLOTS), np.float32)
    iv[rows, slot_of_local] = invc.reshape(NC, NPC)
    ivt = np.ascontiguousarray(
        iv.reshape(NC, WPC, 128).transpose(0, 2, 1))           # [NC, 128, WPC]

    w1x = np.ascontiguousarray(W1[0:F_X], dtype=_BF16)                 # [64,128]
    w1e = np.ascontiguousarray(W1[F_X:F_X + F_E], dtype=_BF16)         # [16,128]
    w1u = np.ascontiguousarray(u @ W1[F_X + F_E:], dtype=_BF16)        # [64,128]
    w2 = np.ascontiguousarray(W2, dtype=_BF16)                         # [128,64]
    b1c = np.ascontiguousarray(b1, np.float32)
    b2c = np.ascontiguousarray(b2, np.float32)
    iotac = np.arange(N_GRAPHS, dtype=np.float32)
    onesc = np.ones(N_GRAPHS, _BF16)
    iotar128 = np.arange(128, dtype=np.float32).astype(_BF16)
    ones128 = np.ones(128, _BF16)
    iotac128 = np.arange(128, dtype=np.float32)
    wbytes = [w1x, w1e, w1u, w2, b1c, b2c, iotac, onesc,
              iotar128, ones128, iotac128]

    stage_caps = [tuple(int(c) for c in caps[k * WPS:(k + 1) * WPS])
                  for k in range(STAGES)]
    in_maps = [{} for _ in range(NC)]
    for k in range(STAGES):
        layout, nbytes = _blob_layout(stage_caps[k])
        e0 = int(offs[k * WPS])
        e1 = e0 + int(sum(stage_caps[k])) * 128 * F_E
        s0, s1 = k * SPS, (k + 1) * SPS
        for ci in range(NC):
            blob = np.zeros(nbytes, np.uint8)

            def put(name, arr):
                bts = np.ascontiguousarray(arr).view(np.uint8).ravel()
                blob[layout[name]:layout[name] + bts.size] = bts

            put("edges", A[ci, e0:e1])
            put("xt", xt[ci][:, s0:s1])
            put("gid", gid[ci][s0:s1])
            put("invc", ivt[ci][:, k * WPS:(k + 1) * WPS])
            for nm, arr in zip(("w1x", "w1e", "w1u", "w2", "b1", "b2",
                                "iota64", "ones", "iotar", "ones128", "iotac"),
                               wbytes):
                put(nm, arr)
            in_maps[ci][f"blob{k}"] = blob

    plan = dict(stage_caps=tuple(stage_caps), slot_of_local=slot_of_local)
    return plan, in_maps


def _postprocess(stage_outs, plan):
    """stage_outs: list of STAGES arrays [NC*GPS, F_OUT, 512] f16."""
    slot_of_local = plan["slot_of_local"]
    out = np.empty((NC * NPC, F_OUT), np.float32)
    o = np.concatenate(
        [so.reshape(NC, GPS, F_OUT, B * 128) for so in stage_outs], axis=1)
    for ci in range(NC):
        o2 = o[ci].transpose(1, 0, 2).reshape(F_OUT, SLOTS)
        out[ci * NPC:(ci + 1) * NPC] = o2[:, slot_of_local[ci]].T
    return out


# ------------------------------------------------------------- device side
def _build_stage(caps_k):
    import concourse.bacc as bacc
    import concourse.mybir as mybir
    import concourse.tile as tile
    from contextlib import ExitStack

    f32 = mybir.dt.float32
    bf16 = mybir.dt.bfloat16
    f16 = mybir.dt.float16
    fp8 = mybir.dt.float8e3
    u8 = mybir.dt.uint8
    AF = mybir.ActivationFunctionType

    caps_k = list(caps_k)
    layout, nbytes = _blob_layout(caps_k)
    eoffs = [0] * WPS
    for w in range(1, WPS):
        eoffs[w] = eoffs[w - 1] + caps_k[w - 1] * 128 * F_E

    nc = bacc.Bacc("TRN2", target_bir_lowering=False)
    blob_d = nc.dram_tensor("blob", [nbytes], u8, kind="ExternalInput")
    out_d = nc.dram_tensor("outT", [GPS, F_OUT, B * 128], f16,
                           kind="ExternalOutput")

    def view(name, dt, p, q):
        o = layout[name]
        sz = p * q * np.dtype(mybir.dt.np(dt)).itemsize
        return blob_d[o:o + sz].bitcast(dt).rearrange("(p q) -> p q", p=p)

    with tile.TileContext(nc) as tc, ExitStack() as ctx:
        consts = ctx.enter_context(tc.tile_pool(name="consts", bufs=1))
        edge_pool = ctx.enter_context(tc.tile_pool(name="edges", bufs=4))
        gsn_pool = ctx.enter_context(tc.tile_pool(name="gsn", bufs=4))
        gsc_pool = ctx.enter_context(tc.tile_pool(name="gsc", bufs=2))
        ea_pool = ctx.enter_context(tc.tile_pool(name="ea", bufs=2))
        hid_pool = ctx.enter_context(tc.tile_pool(name="hid", bufs=2))
        out_pool = ctx.enter_context(tc.tile_pool(name="outs", bufs=3))
        pse_pool = ctx.enter_context(
            tc.tile_pool(name="pse", bufs=2, space="PSUM"))
        psh_pool = ctx.enter_context(
            tc.tile_pool(name="psh", bufs=2, space="PSUM"))
        pso_pool = ctx.enter_context(
            tc.tile_pool(name="pso", bufs=2, space="PSUM"))
        psb_pool = ctx.enter_context(
            tc.tile_pool(name="psb", bufs=1, space="PSUM"))

        w1x_t = consts.tile([F_X, H], bf16)
        nc.sync.dma_start(w1x_t[:], view("w1x", bf16, F_X, H))
        w1e_t = consts.tile([F_E, H], bf16)
        nc.sync.dma_start(w1e_t[:], view("w1e", bf16, F_E, H))
        w1u_t = consts.tile([N_GRAPHS, H], bf16)
        nc.sync.dma_start(w1u_t[:], view("w1u", bf16, N_GRAPHS, H))
        w2_t = consts.tile([H, F_OUT], bf16)
        nc.sync.dma_start(w2_t[:], view("w2", bf16, H, F_OUT))
        b1_t = consts.tile([H, 1], f32)
        nc.sync.dma_start(b1_t[:], view("b1", f32, H, 1))
        b2_t = consts.tile([F_OUT, 1], f32)
        nc.sync.dma_start(b2_t[:], view("b2", f32, F_OUT, 1))
        invc_t = consts.tile([128, WPS], f32)
        nc.sync.dma_start(invc_t[:], view("invc", f32, 128, WPS))

        if XT_FP8:
            xt8_t = consts.tile([F_X, SPS], fp8)
            nc.sync.dma_start(xt8_t[:], view("xt", fp8, F_X, SPS))
            xt_t = consts.tile([F_X, SPS], bf16)
            nc.vector.tensor_copy(xt_t[:], xt8_t[:])
        else:
            xt_t = consts.tile([F_X, SPS], bf16)
            nc.sync.dma_start(xt_t[:], view("xt", bf16, F_X, SPS))
        # identity for PE transpose, built on device: broadcast iota(128) row
        # across partitions with a K=1 matmul, compare against iota column
        iotar_t = consts.tile([1, 128], bf16)
        nc.sync.dma_start(iotar_t[:], view("iotar", bf16, 1, 128))
        ones128_t = consts.tile([1, 128], bf16)
        nc.sync.dma_start(ones128_t[:], view("ones128", bf16, 1, 128))
        iotac_t = consts.tile([128, 1], f32)
        nc.sync.dma_start(iotac_t[:], view("iotac", f32, 128, 1))
        ident_t = consts.tile([128, 128], f32)
        psi = psb_pool.tile([128, 128], f32)
        nc.tensor.matmul(psi[:], ones128_t[:], iotar_t[:], start=True, stop=True)
        nc.vector.tensor_scalar(
            out=ident_t[:], in0=psi[:], scalar1=iotac_t[:], scalar2=None,
            op0=mybir.AluOpType.is_equal,
        )

        # one-hot graph membership built on device: broadcast gid across 64
        # partitions with a K=1 matmul, then compare against iota(64)
        gid_t = consts.tile([1, SPS], bf16)
        nc.sync.dma_start(gid_t[:], view("gid", bf16, 1, SPS))
        iota_t = consts.tile([N_GRAPHS, 1], f32)
        nc.sync.dma_start(iota_t[:], view("iota64", f32, N_GRAPHS, 1))
        ones_t = consts.tile([1, N_GRAPHS], bf16)
        nc.sync.dma_start(ones_t[:], view("ones", bf16, 1, N_GRAPHS))
        gidb_t = consts.tile([N_GRAPHS, SPS], f32)
        ohb_t = consts.tile([N_GRAPHS, SPS], bf16)
        for c in range(SPS // 512):
            psb = psb_pool.tile([N_GRAPHS, 512], f32)
            nc.tensor.matmul(psb[:], ones_t[:], gid_t[:, c * 512:(c + 1) * 512],
                             start=True, stop=True)
            nc.vector.tensor_copy(gidb_t[:, c * 512:(c + 1) * 512], psb[:])
        nc.vector.tensor_scalar(
            out=ohb_t[:], in0=gidb_t[:], scalar1=iota_t[:], scalar2=None,
            op0=mybir.AluOpType.is_equal,
        )

        eb = layout["edges"]
        for g in range(GPS):
            gsc = gsc_pool.tile([128, B * F_E], f32)
            for j in range(B):
                w = g * B + j
                cw = caps_k[w]
                et = edge_pool.tile([128, F_E * cw], fp8)
                src = blob_d[eb + eoffs[w]:eb + eoffs[w] + 128 * F_E * cw]
                nc.sync.dma_start(
                    et[:], src.bitcast(fp8).rearrange("(p q) -> p q", p=128))
                gsn = gsn_pool.tile([128, F_E], f32)
                nc.vector.tensor_reduce(
                    out=gsn[:],
                    in_=et[:].rearrange("p (f e) -> p f e", e=cw),
                    axis=mybir.AxisListType.X,
                    op=mybir.AluOpType.add,
                )
                nc.scalar.activation(
                    gsc[:, j * F_E:(j + 1) * F_E], gsn[:], AF.Identity,
                    scale=invc_t[:, w:w + 1],
                )

            pse = pse_pool.tile([F_E, B * 128], f32)
            for j in range(B):
                nc.tensor.transpose(
                    pse[:, j * 128:(j + 1) * 128],
                    gsc[:, j * F_E:(j + 1) * F_E],
                    ident_t[:],
                )
            ea = ea_pool.tile([F_E, B * 128], bf16)
            nc.vector.tensor_copy(ea[:], pse[:])

            psh = psh_pool.tile([H, B * 128], f32)
            nc.tensor.matmul(psh[:], w1e_t[:], ea[:], start=True, stop=False)
            nc.tensor.matmul(psh[:], w1x_t[:],
                             xt_t[:, g * 512:(g + 1) * 512],
                             start=False, stop=False)
            nc.tensor.matmul(psh[:], w1u_t[:],
                             ohb_t[:, g * 512:(g + 1) * 512],
                             start=False, stop=True)
            hid = hid_pool.tile([H, B * 128], bf16)
            nc.scalar.activation(hid[:], psh[:], AF.Relu, bias=b1_t[:])

            pso = pso_pool.tile([F_OUT, B * 128], f32)
            nc.tensor.matmul(pso[:], w2_t[:], hid[:], start=True, stop=True)
            outs = out_pool.tile([F_OUT, B * 128], f16)
            nc.scalar.activation(outs[:], pso[:], AF.Identity, bias=b2_t[:])
            nc.sync.dma_start(out_d[g], outs[:])

    nc.finalize()
    return nc


def _get_program(caps_k):
    key = (caps_k, XT_FP8)
    if key not in _PROGRAM_CACHE:
        _PROGRAM_CACHE[key] = _build_stage(caps_k)
    return _PROGRAM_CACHE[key]


# ------------------------------------------------------------- runner
class _Stage:
    def __init__(self, nc, sharding, mesh, jax, bass2jax, mybir):
        partition_name = (nc.partition_id_tensor.name
                          if nc.partition_id_tensor else None)
        in_names, out_names, out_avals = [], [], []
        for alloc in nc.m.functions[0].allocations:
            if not isinstance(alloc, mybir.MemoryLocationSet):
                continue
            name = alloc.memorylocations[0].name
            if alloc.kind == "ExternalInput":
                if name != partition_name:
                    in_names.append(name)
            elif alloc.kind == "ExternalOutput":
                out_names.append(name)
                out_avals.append(jax.core.ShapedArray(
                    tuple(alloc.tensor_shape), mybir.dt.np(alloc.dtype)))
        in_names_all = list(in_names) + out_names
        if partition_name is not None:
            in_names_all.append(partition_name)

        def _body(*args):
            operands = list(args)
            if partition_name is not None:
                operands.append(bass2jax.partition_id_tensor())
            return tuple(bass2jax._bass_exec_p.bind(
                *operands,
                out_avals=tuple(out_avals),
                in_names=tuple(in_names_all),
                out_names=tuple(out_names),
                lowering_input_output_aliases=(),
                sim_require_finite=True,
                sim_require_nnan=True,
                nc=nc,
            ))

        from jax.sharding import PartitionSpec
        from jax.experimental.shard_map import shard_map
        n_ops = len(in_names) + len(out_names)
        self.sharded = jax.jit(
            shard_map(_body, mesh=mesh,
                      in_specs=(PartitionSpec("core"),) * n_ops,
                      out_specs=(PartitionSpec("core"),) * len(out_names),
                      check_rep=False),
            keep_unused=True,
        )
        # output-named dummy operands: never read by the NEFF (the kernel
        # writes every output element), device-resident, reused every call
        self.dummy_outs = [
            jax.device_put(np.zeros((NC * a.shape[0], *a.shape[1:]), a.dtype),
                           sharding)
            for a in out_avals
        ]


class _Runner:
    def __init__(self, plan):
        import jax
        from jax.sharding import Mesh, PartitionSpec, NamedSharding
        import concourse.bass2jax as bass2jax
        import concourse.mybir as mybir

        self.jax = jax
        bass2jax.install_neuronx_cc_hook()
        devices = jax.devices()[:NC]
        mesh = Mesh(np.asarray(devices), ("core",))
        self.sharding = NamedSharding(mesh, PartitionSpec("core"))
        self.stages = [
            _Stage(_get_program(plan["stage_caps"][k]), self.sharding, mesh,
                   jax, bass2jax, mybir)
            for k in range(STAGES)
        ]
        self.pool = ThreadPoolExecutor(2)
        self.jax.block_until_ready(
            [d for s in self.stages for d in s.dummy_outs])

    def call(self, in_maps):
        """Preprocessed per-core blobs -> per-stage host output arrays."""
        futs = []
        for k in range(STAGES):
            g = np.concatenate([in_maps[ci][f"blob{k}"] for ci in range(NC)])
            d = self.jax.device_put(g, self.sharding)           # async H2D
            o = self.stages[k].sharded(d, *self.stages[k].dummy_outs)
            futs.append(self.pool.submit(np.asarray, o[0]))     # D2H thread
        return [f.result() for f in futs]


def _get_runner(plan):
    key = (plan["stage_caps"], XT_FP8)
    if key not in _RUNNER_CACHE:
        _RUNNER_CACHE[key] = _Runner(plan)
    return _RUNNER_CACHE[key]


def run(inputs):
    plan, in_maps = _plan_and_preprocess(inputs)
    runner = _get_runner(plan)
    stage_outs = runner.call(in_maps)
    return _postprocess(stage_outs, plan), plan, in_maps, runner


def kernel(**inputs):
    return run(inputs)[0]
